# revision 11
# baseline (speedup 1.0000x reference)
"""Trainium2 Bass kernel for attention pooling (nn_AttentionLayer).

Reference math (per batch b):
    score  = tanh(x @ W + b)        # [S, D]
    logits = score @ V              # [S, 1]
    attn   = softmax(logits, axis=S)
    out    = sum_s attn[s] * x[s]   # [D]

Sharding: data-parallel over batch across 8 NeuronCores (4 batches/core).
W/b/V replicated. No collectives.

Per-core dataflow (B_LOC=4, S=4096, D=256; seq chunks of SC=1024, folded
s = s0 + p*8 + f so each partition's source rows are contiguous):
  1. SWDGE cast-DMA HBM->SBUF f32->bf16: x_nat[p, f, d] per chunk.  The
     first two chunks are split into fold-halves to shorten pipeline fill.
  2. xT[d_low, (f,dc), s_low] built two ways, load-balanced across devices:
     some chunks via the DMA xbar transpose, the rest on the PE
     (is_transpose matmuls into a PSUM bank, DVE copy back to SBUF).
  3. scoreT[e, s] = W.T @ x.T on TensorE (W stationary, xT moving),
     PSUM pair tiles [128, 2(ec), 512].
  4. one ACT tanh per pair -> st bf16 (b==0 per the problem spec, so both
     ec halves merge into a single instruction; a safe per-ec-bias build
     is compiled on demand if b is ever nonzero).
  5. logits via fat-stationary/skinny-moving matmuls: stationary =
     st[:, ec, 128-col block], moving = V chunk [128, 1] -> PSUM column.
     This lands logitsT [s_low, s_hi] directly in natural layout (PL
     region of the shared MISC psum bank) - no collect/scatter/transpose.
  6. one ACT exp per batch (PSUM -> elog bf16, accum_out -> denominator
     partials per partition; host sums 128 values).
  7. numerator via the same trick: stationary = x_nat[:, f, dc*128:...],
     moving = elog column [128, 1], accumulated into MISC NUM columns.
     Ldweights are free and 1-col matmuls cost ~1 PE cycle each.
  8. tiny DMAs out: num [128, 10], acc [128, 5]; host does the divide.
The last batch's exp/numerator is split in halves so half a overlaps the
final score groups, shrinking the serial tail.
"""

import os
import sys

import numpy as np

_TRN_REPO = "/opt/trn_rl_repo"

B, S, D = 32, 4096, 256
N_CORES = 8
B_LOC = B // N_CORES          # 4 batches per core
SC = 1024                     # seq chunk
F = SC // 128                 # folds per chunk (8); s = s0 + p*F + f
NCH = S // SC                 # chunks per batch (4)
NCHT = B_LOC * NCH            # chunks per core (16)
NSLOT = 2 * NCHT              # score groups (512 seqs) per core (32)

# chunks whose transpose runs on the PE (+DVE copy-out) instead of the DMA
# xbar; picked to cover the pipeline fill (0,1) and to offload the DMA device
PE_T = {0, 1, 9, 10, 11, 12, 13, 14, 15}
N_SPIN = 30                   # PE warm-up spin matmuls (256 cols each)

_cache = {}


def _build(use_bias=False):
    sys.path.insert(0, _TRN_REPO)
    import concourse.bacc as bacc
    import concourse.tile as tile
    from concourse import mybir

    f32 = mybir.dt.float32
    bf16 = mybir.dt.bfloat16

    nc = bacc.Bacc("TRN2", target_bir_lowering=False, debug=False)

    x_d = nc.dram_tensor("inputs", (B_LOC, S, D), f32, kind="ExternalInput")
    W_d = nc.dram_tensor("W", (D, D), f32, kind="ExternalInput")
    b_d = nc.dram_tensor("b", (D,), f32, kind="ExternalInput")
    V_d = nc.dram_tensor("V", (D, 1), f32, kind="ExternalInput")
    num_d = nc.dram_tensor("num", (128, 10), f32, kind="ExternalOutput")
    acc_d = nc.dram_tensor("acc", (128, 5), f32, kind="ExternalOutput")

    with tile.TileContext(nc) as tc:
        with (
            tc.tile_pool(name="consts", bufs=1) as consts,
            tc.tile_pool(name="xpool", bufs=NCHT) as xpool,
            tc.tile_pool(name="xtpool", bufs=NCHT) as xtpool,
            tc.tile_pool(name="stpool", bufs=4) as stpool,
            tc.tile_pool(name="smalls", bufs=1) as smalls,
            tc.tile_pool(name="pspool", bufs=3, space="PSUM") as pspool,
            tc.tile_pool(name="trpool", bufs=1, space="PSUM") as trpool,
            tc.tile_pool(name="miscpool", bufs=1, space="PSUM") as miscpool,
        ):
            # ---- dependency-free prologue first: DVE memsets + Pool
            #      identity build, so the PE transposes and warm-up spins
            #      aren't stuck behind loads/casts in those queues ----
            ones_sb = consts.tile([128, 128], bf16)
            nc.vector.memset(ones_sb, 1.0)
            dummy_sb = consts.tile([128, 2], bf16)
            nc.vector.memset(dummy_sb, 0.0)
            dummy_mov = consts.tile([128, 256], bf16)
            nc.vector.memset(dummy_mov, 0.0)
            zero_bias = consts.tile([128, 1], f32)
            nc.vector.memset(zero_bias, 0.0)
            I_sb = consts.tile([128, 128], bf16)
            nc.gpsimd.affine_select(
                out=I_sb,
                in_=ones_sb,
                pattern=[[-1, 128]],
                compare_op=mybir.AluOpType.is_equal,
                fill=0.0,
                base=0,
                channel_multiplier=1,
            )

            # ---- early DMA issue: params on HWDGE/SP, x chunks on SWDGE ----
            W_f = consts.tile([128, 2, D], f32)
            nc.sync.dma_start(
                out=W_f, in_=W_d[:, :].rearrange("(dc p) e -> p dc e", p=128)
            )
            V_f = consts.tile([128, 2], f32)
            nc.sync.dma_start(
                out=V_f, in_=V_d[:, :].rearrange("(ec p) o -> p (ec o)", p=128)
            )
            b_sb = consts.tile([128, 2], f32)
            nc.sync.dma_start(
                out=b_sb, in_=b_d[:].rearrange("(ec p) -> p ec", p=128)
            )

            x_nat = {}

            def load(ci, half=None):
                bb, c = divmod(ci, NCH)
                s0 = c * SC
                if half is None:
                    t = xpool.tile([128, F, D], bf16, name="x_nat", tag="x")
                    src = x_d[bb, s0 : s0 + SC, :].rearrange(
                        "(p f) d -> p f d", p=128
                    )
                    nc.gpsimd.dma_start(out=t, in_=src)
                    x_nat[ci] = t
                else:
                    # fold-half load: partitions keep contiguous source rows
                    if half == 0:
                        t = xpool.tile([128, F, D], bf16, name="x_nat", tag="x")
                        x_nat[ci] = t
                    t = x_nat[ci]
                    fh = F // 2
                    src = x_d[bb, s0 : s0 + SC, :].rearrange(
                        "(p f) d -> p f d", p=128
                    )[:, half * fh : (half + 1) * fh, :]
                    nc.gpsimd.dma_start(
                        out=t[:, half * fh : (half + 1) * fh, :], in_=src
                    )

            xT = {}

            def dma_transpose(ci):
                t = xtpool.tile([128, 2 * F, 128], bf16, name="xT", tag="xT")
                nc.sync.dma_start(out=t, in_=x_nat[ci], transpose=True)
                xT[ci] = t

            def alloc_xt(ci):
                xT[ci] = xtpool.tile([128, 2 * F, 128], bf16, name="xT", tag="xT")

            def pe_transpose_half(ci, h):
                # 4 folds x 2 d-chunks -> 8 [128,128] tiles -> 1 psum bank
                tr = trpool.tile([128, 1024], bf16, name="tr", tag="tr")
                for t in range(8):
                    f = 4 * h + t // 2
                    dc = t % 2
                    nc.tensor.transpose(
                        out=tr[:, t * 128 : (t + 1) * 128],
                        in_=x_nat[ci][:, f, dc * 128 : (dc + 1) * 128],
                        identity=I_sb,
                    )
                nc.vector.tensor_copy(
                    out=xT[ci][:, 8 * h : 8 * h + 8, :],
                    in_=tr.rearrange("p (a b) -> p a b", a=8),
                )

            # ---- param casts ----
            W_sb = consts.tile([128, 2, D], bf16)
            nc.vector.tensor_copy(out=W_sb, in_=W_f)
            V_sb = consts.tile([128, 2], bf16)
            nc.vector.tensor_copy(out=V_sb, in_=V_f)

            # shared psum bank: PL logits cols 0..127, NUM cols 128..137,
            # spin target cols 256..511
            MISC = miscpool.tile([128, 512], f32, name="MISC")

            def spin(n):
                for _ in range(n):
                    nc.tensor.matmul(
                        MISC[0:2, 256:512],
                        dummy_sb,
                        dummy_mov,
                        start=True,
                        stop=True,
                    )

            # all loads issued up-front (chunks 0,1 as fold-halves for fast
            # fill); SBUF holds every chunk so nothing gates on pool reuse,
            # and the DMA device stays saturated back-to-back
            load(0, half=0)
            load(0, half=1)
            load(1, half=0)
            load(1, half=1)
            for ci in range(2, NCHT):
                load(ci)
            for ci in range(NCHT):
                if ci in PE_T:
                    alloc_xt(ci)
                else:
                    dma_transpose(ci)

            spin(N_SPIN)

            # ---- outputs / softmax state ----
            num_sb = smalls.tile([128, 10], f32, name="num_sb")
            acc_sb = smalls.tile([128, 5], f32, name="acc_sb")
            elog = {}

            st_tiles = {}

            def score_group(q):
                ci, g = divmod(q, 2)
                ps = pspool.tile([128, 2, 512], f32, name="ps", tag="ps")
                xv = xT[ci].rearrange("p (f dc) s -> p f dc s", dc=2)
                for ec in range(2):
                    for dc in range(2):
                        nc.tensor.matmul(
                            ps[:, ec, :],
                            W_sb[:, dc, ec * 128 : (ec + 1) * 128],
                            xv[:, 4 * g : 4 * g + 4, dc, :],
                            start=(dc == 0),
                            stop=(dc == 1),
                        )
                st = stpool.tile([128, 2, 512], bf16, name="st", tag="st")
                if use_bias:
                    for ec in range(2):
                        nc.scalar.activation(
                            out=st[:, ec, :],
                            in_=ps[:, ec, :],
                            func=mybir.ActivationFunctionType.Tanh,
                            bias=b_sb[:, ec : ec + 1],
                            scale=1.0,
                        )
                else:
                    nc.scalar.activation(
                        out=st,
                        in_=ps,
                        func=mybir.ActivationFunctionType.Tanh,
                        bias=zero_bias[:, 0:1],
                        scale=1.0,
                    )
                st_tiles[q] = st

            def logits_group(q):
                ci, g = divmod(q, 2)
                bb, c = divmod(ci, NCH)
                st = st_tiles.pop(q)
                for k in range(4):
                    col = bb * 32 + c * 8 + g * 4 + k
                    for ec in range(2):
                        nc.tensor.matmul(
                            MISC[:, col : col + 1],
                            st[:, ec, k * 128 : (k + 1) * 128],
                            V_sb[:, ec : ec + 1],
                            start=(ec == 0),
                            stop=(ec == 1),
                        )

            def exp_batch(bb, half=None):
                if bb not in elog:
                    elog[bb] = smalls.tile(
                        [128, 32], bf16, name="elog", tag="elog", bufs=2
                    )
                if half is None:
                    src = MISC[:, bb * 32 : bb * 32 + 32]
                    dst = elog[bb]
                    acc = acc_sb[:, bb : bb + 1]
                else:
                    src = MISC[:, bb * 32 + 16 * half : bb * 32 + 16 * (half + 1)]
                    dst = elog[bb][:, 16 * half : 16 * (half + 1)]
                    acc = acc_sb[:, bb + half : bb + half + 1]
                nc.scalar.activation(
                    out=dst,
                    in_=src,
                    func=mybir.ActivationFunctionType.Exp,
                    accum_out=acc,
                )

            def num_block(bb, half, numcol0):
                # accumulate numerator into MISC cols numcol0 (dc=0) /
                # numcol0+1 (dc=1); half=None covers the whole batch in one
                # accumulation group, half=0/1 cover two chunks each (the
                # last batch uses separate cols per half, merged on host)
                if half is None:
                    cis = [4 * bb + j for j in range(4)]
                else:
                    cis = [4 * bb + 2 * half, 4 * bb + 2 * half + 1]
                for dc in range(2):
                    first = True
                    for ci in cis:
                        c = ci % NCH
                        for f in range(F):
                            nc.tensor.matmul(
                                MISC[:, numcol0 + dc : numcol0 + dc + 1],
                                x_nat[ci][:, f, dc * 128 : (dc + 1) * 128],
                                elog[bb][:, c * 8 + f : c * 8 + f + 1],
                                start=first,
                                stop=(ci == cis[-1] and f == F - 1),
                            )
                            first = False

            def num_copy(numcol0, outcol0, n=2):
                nc.vector.tensor_copy(
                    out=num_sb[:, outcol0 : outcol0 + n],
                    in_=MISC[:, numcol0 : numcol0 + n],
                )

            # ---- main software-pipelined slot loop ----
            # PE order per slot q: [pe-transpose half], score(q),
            # logits(q-2), [numerator of batch finished 2 slots ago]
            pe_t_pending = [
                (ci, h) for ci in range(NCHT) if ci in PE_T for h in (0, 1)
            ]

            for q in range(NSLOT):
                ci, g = divmod(q, 2)
                bb = ci // NCH
                # at most one PE transpose half-block per slot, spaced so the
                # single-bank psum ping works off the DVE copy WAR sems
                if pe_t_pending:
                    tci, th = pe_t_pending.pop(0)
                    pe_transpose_half(tci, th)
                score_group(q)
                if q >= 2:
                    logits_group(q - 2)
                    qq = q - 2
                    if qq % 8 == 7 and qq // 8 < 3:
                        # batch bbq fully projected -> exp now, numerator
                        # two slots later (gives the ACT queue time)
                        exp_batch(qq // 8)
                if q >= 4 and (q - 4) % 8 == 7:
                    bbq = (q - 4) // 8
                    if bbq < 3:
                        num_block(bbq, None, 128 + 2 * bbq)
                        num_copy(128 + 2 * bbq, 2 * bbq)
                # last batch, first half: groups 24..27 projected once
                # logits(27) emitted at slot 29 -> exp half at 29, num at 31
                if q == 29:
                    exp_batch(3, half=0)
                if q == 31:
                    num_block(3, 0, 134)
                    num_copy(134, 6)

            # ---- tail: last two logit groups, second exp half, numerator;
            #      outputs already computed stream out early so only the
            #      last slivers sit on the critical path ----
            logits_group(30)
            nc.sync.dma_start(out=num_d[:, 0:8], in_=num_sb[:, 0:8])
            nc.scalar.dma_start(out=acc_d[:, 0:4], in_=acc_sb[:, 0:4])
            spin(4)
            logits_group(31)
            exp_batch(3, half=1)
            spin(5)
            num_block(3, 1, 136)
            num_copy(136, 8)

            nc.sync.dma_start(out=num_d[:, 8:10], in_=num_sb[:, 8:10])
            nc.scalar.dma_start(out=acc_d[:, 4:5], in_=acc_sb[:, 4:5])

    nc.compile()
    return nc


def _get_nc(use_bias=False):
    key = "nc_bias" if use_bias else "nc"
    if key not in _cache:
        _cache[key] = _build(use_bias)
    return _cache[key]


def kernel(inputs, W, b, V):
    sys.path.insert(0, _TRN_REPO)
    from concourse.bass_utils import run_bass_kernel_spmd

    inputs = np.ascontiguousarray(np.asarray(inputs, dtype=np.float32))
    W = np.ascontiguousarray(np.asarray(W, dtype=np.float32))
    b = np.ascontiguousarray(np.asarray(b, dtype=np.float32))
    V = np.ascontiguousarray(np.asarray(V, dtype=np.float32))

    # the fast build fuses tanh across both e-chunks, which requires b == 0
    # (guaranteed by the problem spec); fall back to a per-ec-bias build if
    # a nonzero bias ever shows up
    use_bias = bool(np.any(b != 0.0))
    nc = _get_nc(use_bias)

    in_maps = [
        {
            "inputs": inputs[i * B_LOC : (i + 1) * B_LOC],
            "W": W,
            "b": b,
            "V": V,
        }
        for i in range(N_CORES)
    ]

    trace = bool(int(os.environ.get("BENCH_TRACE", "0")))
    try:
        res = run_bass_kernel_spmd(
            nc, in_maps, core_ids=list(range(N_CORES)), trace=trace
        )
    except ModuleNotFoundError:
        res = run_bass_kernel_spmd(
            nc, in_maps, core_ids=list(range(N_CORES)), trace=False
        )
    _cache["last_exec_time_ns"] = res.exec_time_ns
    _cache["last_result"] = res
    outs = []
    for r in res.results:
        num = r["num"]          # [128, 10]
        acc = r["acc"]          # [128, 5]
        ctx = np.empty((B_LOC, D), dtype=np.float32)
        for bb in range(3):
            den = acc[:, bb].sum()
            ctx[bb, :128] = num[:, 2 * bb] / den
            ctx[bb, 128:] = num[:, 2 * bb + 1] / den
        den3 = acc[:, 3].sum() + acc[:, 4].sum()
        ctx[3, :128] = (num[:, 6] + num[:, 8]) / den3
        ctx[3, 128:] = (num[:, 7] + num[:, 9]) / den3
        outs.append(ctx)
    return np.concatenate(outs, axis=0)


# revision 12
# speedup vs baseline: 1.1197x; 1.1197x over previous
"""Trainium2 Bass kernel for attention pooling (nn_AttentionLayer).

Reference math (per batch b):
    score  = tanh(x @ W + b)        # [S, D]
    logits = score @ V              # [S, 1]
    attn   = softmax(logits, axis=S)
    out    = sum_s attn[s] * x[s]   # [D]

Sharding: data-parallel over batch across 8 NeuronCores (4 batches/core).
W/b/V replicated. No collectives.

Per-core dataflow (B_LOC=4, S=4096, D=256; seq chunks of SC=1024, folded
s = s0 + p*8 + f so each partition's source rows are contiguous):
  1. SWDGE cast-DMA HBM->SBUF f32->bf16: x_nat[p, f, d] per chunk.  The
     first two chunks are split into fold-halves to shorten pipeline fill.
  2. xT[d_low, (f,dc), s_low] built two ways, load-balanced across devices:
     some chunks via the DMA xbar transpose, the rest on the PE
     (is_transpose matmuls into a PSUM bank, DVE copy back to SBUF).
  3. scoreT[e, s] = W.T @ x.T on TensorE (W stationary, xT moving),
     PSUM pair tiles [128, 2(ec), 512].
  4. one ACT tanh per pair -> st bf16 (b==0 per the problem spec, so both
     ec halves merge into a single instruction; a safe per-ec-bias build
     is compiled on demand if b is ever nonzero).
  5. logits via fat-stationary/skinny-moving matmuls: stationary =
     st[:, ec, 128-col block], moving = V chunk [128, 1] -> PSUM column.
     This lands logitsT [s_low, s_hi] directly in natural layout (PL
     region of the shared MISC psum bank) - no collect/scatter/transpose.
  6. one ACT exp per batch (PSUM -> elog bf16, accum_out -> denominator
     partials per partition; host sums 128 values).
  7. numerator via the same trick: stationary = x_nat[:, f, dc*128:...],
     moving = elog column [128, 1], accumulated into MISC NUM columns.
     Ldweights are free and 1-col matmuls cost ~1 PE cycle each.
  8. tiny DMAs out: num [128, 10], acc [128, 5]; host does the divide.
The last batch's exp/numerator is split in halves so half a overlaps the
final score groups, shrinking the serial tail.
"""

import os
import sys

import numpy as np

_TRN_REPO = "/opt/trn_rl_repo"

B, S, D = 32, 4096, 256
N_CORES = 8
B_LOC = B // N_CORES          # 4 batches per core
SC = 1024                     # seq chunk
F = SC // 128                 # folds per chunk (8); s = s0 + p*F + f
NCH = S // SC                 # chunks per batch (4)
NCHT = B_LOC * NCH            # chunks per core (16)
NSLOT = 2 * NCHT              # score groups (512 seqs) per core (32)

# chunks whose transpose runs on the PE (+DVE copy-out) instead of the DMA
# xbar; picked to cover the pipeline fill (0,1) and to offload the DMA device
PE_T = {0, 1, 9, 10, 11, 12, 13, 14, 15}
N_SPIN = 30                   # PE warm-up spin matmuls (256 cols each)

_cache = {}


def _build(use_bias=False):
    sys.path.insert(0, _TRN_REPO)
    import concourse.bacc as bacc
    import concourse.tile as tile
    from concourse import mybir

    f32 = mybir.dt.float32
    bf16 = mybir.dt.bfloat16

    nc = bacc.Bacc("TRN2", target_bir_lowering=False, debug=False)

    x_d = nc.dram_tensor("inputs", (B_LOC, S, D), f32, kind="ExternalInput")
    W_d = nc.dram_tensor("W", (D, D), f32, kind="ExternalInput")
    b_d = nc.dram_tensor("b", (D,), f32, kind="ExternalInput")
    V_d = nc.dram_tensor("V", (D, 1), f32, kind="ExternalInput")
    num_d = nc.dram_tensor("num", (128, 10), f32, kind="ExternalOutput")
    acc_d = nc.dram_tensor("acc", (128, 5), f32, kind="ExternalOutput")

    with tile.TileContext(nc) as tc:
        with (
            tc.tile_pool(name="consts", bufs=1) as consts,
            tc.tile_pool(name="xpool", bufs=NCHT) as xpool,
            tc.tile_pool(name="xtpool", bufs=NCHT) as xtpool,
            tc.tile_pool(name="stpool", bufs=4) as stpool,
            tc.tile_pool(name="smalls", bufs=1) as smalls,
            tc.tile_pool(name="pspool", bufs=3, space="PSUM") as pspool,
            tc.tile_pool(name="trpool", bufs=1, space="PSUM") as trpool,
            tc.tile_pool(name="miscpool", bufs=1, space="PSUM") as miscpool,
        ):
            # ---- dependency-free prologue first: DVE memsets + Pool
            #      identity build, so the PE transposes and warm-up spins
            #      aren't stuck behind loads/casts in those queues ----
            ones_sb = consts.tile([128, 128], bf16)
            nc.vector.memset(ones_sb, 1.0)
            dummy_sb = consts.tile([128, 2], bf16)
            nc.vector.memset(dummy_sb, 0.0)
            dummy_mov = consts.tile([128, 256], bf16)
            nc.vector.memset(dummy_mov, 0.0)
            zero_bias = consts.tile([128, 1], f32)
            nc.vector.memset(zero_bias, 0.0)
            I_sb = consts.tile([128, 128], bf16)
            nc.gpsimd.affine_select(
                out=I_sb,
                in_=ones_sb,
                pattern=[[-1, 128]],
                compare_op=mybir.AluOpType.is_equal,
                fill=0.0,
                base=0,
                channel_multiplier=1,
            )

            # ---- early DMA issue: params on HWDGE/SP, x chunks on SWDGE ----
            W_f = consts.tile([128, 2, D], f32)
            nc.sync.dma_start(
                out=W_f, in_=W_d[:, :].rearrange("(dc p) e -> p dc e", p=128)
            )
            V_f = consts.tile([128, 2], f32)
            nc.sync.dma_start(
                out=V_f, in_=V_d[:, :].rearrange("(ec p) o -> p (ec o)", p=128)
            )
            b_sb = consts.tile([128, 2], f32)
            nc.sync.dma_start(
                out=b_sb, in_=b_d[:].rearrange("(ec p) -> p ec", p=128)
            )

            x_nat = {}

            def load(ci, half=None):
                bb, c = divmod(ci, NCH)
                s0 = c * SC
                if half is None:
                    t = xpool.tile([128, F, D], bf16, name="x_nat", tag="x")
                    src = x_d[bb, s0 : s0 + SC, :].rearrange(
                        "(p f) d -> p f d", p=128
                    )
                    nc.gpsimd.dma_start(out=t, in_=src)
                    x_nat[ci] = t
                else:
                    # fold-half load: partitions keep contiguous source rows
                    if half == 0:
                        t = xpool.tile([128, F, D], bf16, name="x_nat", tag="x")
                        x_nat[ci] = t
                    t = x_nat[ci]
                    fh = F // 2
                    src = x_d[bb, s0 : s0 + SC, :].rearrange(
                        "(p f) d -> p f d", p=128
                    )[:, half * fh : (half + 1) * fh, :]
                    nc.gpsimd.dma_start(
                        out=t[:, half * fh : (half + 1) * fh, :], in_=src
                    )

            xT = {}

            def dma_transpose(ci):
                t = xtpool.tile([128, 2 * F, 128], bf16, name="xT", tag="xT")
                nc.sync.dma_start(out=t, in_=x_nat[ci], transpose=True)
                xT[ci] = t

            def alloc_xt(ci):
                xT[ci] = xtpool.tile([128, 2 * F, 128], bf16, name="xT", tag="xT")

            def pe_transpose_half(ci, h):
                # 4 folds x 2 d-chunks -> 8 [128,128] tiles -> 1 psum bank
                tr = trpool.tile([128, 1024], bf16, name="tr", tag="tr")
                for t in range(8):
                    f = 4 * h + t // 2
                    dc = t % 2
                    nc.tensor.transpose(
                        out=tr[:, t * 128 : (t + 1) * 128],
                        in_=x_nat[ci][:, f, dc * 128 : (dc + 1) * 128],
                        identity=I_sb,
                    )
                nc.vector.tensor_copy(
                    out=xT[ci][:, 8 * h : 8 * h + 8, :],
                    in_=tr.rearrange("p (a b) -> p a b", a=8),
                )

            # ---- param casts ----
            W_sb = consts.tile([128, 2, D], bf16)
            nc.vector.tensor_copy(out=W_sb, in_=W_f)
            V_sb = consts.tile([128, 2], bf16)
            nc.vector.tensor_copy(out=V_sb, in_=V_f)

            # shared psum bank: PL logits cols 0..127, NUM cols 128..137,
            # spin target cols 256..511
            MISC = miscpool.tile([128, 512], f32, name="MISC")

            def spin(n):
                for _ in range(n):
                    nc.tensor.matmul(
                        MISC[0:2, 256:512],
                        dummy_sb,
                        dummy_mov,
                        start=True,
                        stop=True,
                    )

            # all loads issued up-front (chunks 0,1 as fold-halves for fast
            # fill); SBUF holds every chunk so nothing gates on pool reuse,
            # and the DMA device stays saturated back-to-back
            load(0, half=0)
            load(0, half=1)
            load(1, half=0)
            load(1, half=1)
            for ci in range(2, NCHT):
                load(ci)
            for ci in range(NCHT):
                if ci in PE_T:
                    alloc_xt(ci)
                else:
                    dma_transpose(ci)

            spin(N_SPIN)

            # ---- outputs / softmax state ----
            num_sb = smalls.tile([128, 10], f32, name="num_sb")
            acc_sb = smalls.tile([128, 5], f32, name="acc_sb")
            elog = {}

            st_tiles = {}

            def score_group(q):
                ci, g = divmod(q, 2)
                ps = pspool.tile([128, 2, 512], f32, name="ps", tag="ps")
                xv = xT[ci].rearrange("p (f dc) s -> p f dc s", dc=2)
                for ec in range(2):
                    for dc in range(2):
                        nc.tensor.matmul(
                            ps[:, ec, :],
                            W_sb[:, dc, ec * 128 : (ec + 1) * 128],
                            xv[:, 4 * g : 4 * g + 4, dc, :],
                            start=(dc == 0),
                            stop=(dc == 1),
                        )
                st = stpool.tile([128, 2, 512], bf16, name="st", tag="st")
                if use_bias:
                    for ec in range(2):
                        nc.scalar.activation(
                            out=st[:, ec, :],
                            in_=ps[:, ec, :],
                            func=mybir.ActivationFunctionType.Tanh,
                            bias=b_sb[:, ec : ec + 1],
                            scale=1.0,
                        )
                else:
                    nc.scalar.activation(
                        out=st,
                        in_=ps,
                        func=mybir.ActivationFunctionType.Tanh,
                        bias=zero_bias[:, 0:1],
                        scale=1.0,
                    )
                st_tiles[q] = st

            def logits_group(q):
                ci, g = divmod(q, 2)
                bb, c = divmod(ci, NCH)
                st = st_tiles.pop(q)
                for k in range(4):
                    col = bb * 32 + c * 8 + g * 4 + k
                    for ec in range(2):
                        nc.tensor.matmul(
                            MISC[:, col : col + 1],
                            st[:, ec, k * 128 : (k + 1) * 128],
                            V_sb[:, ec : ec + 1],
                            start=(ec == 0),
                            stop=(ec == 1),
                        )

            def exp_batch(bb, half=None):
                if bb not in elog:
                    elog[bb] = smalls.tile(
                        [128, 32], bf16, name="elog", tag="elog", bufs=2
                    )
                if half is None:
                    src = MISC[:, bb * 32 : bb * 32 + 32]
                    dst = elog[bb]
                    acc = acc_sb[:, bb : bb + 1]
                else:
                    src = MISC[:, bb * 32 + 16 * half : bb * 32 + 16 * (half + 1)]
                    dst = elog[bb][:, 16 * half : 16 * (half + 1)]
                    acc = acc_sb[:, bb + half : bb + half + 1]
                nc.scalar.activation(
                    out=dst,
                    in_=src,
                    func=mybir.ActivationFunctionType.Exp,
                    accum_out=acc,
                )

            def num_block(bb, half, numcol0):
                # accumulate numerator into MISC cols numcol0 (dc=0) /
                # numcol0+1 (dc=1); half=None covers the whole batch in one
                # accumulation group, half=0/1 cover two chunks each (the
                # last batch uses separate cols per half, merged on host)
                if half is None:
                    cis = [4 * bb + j for j in range(4)]
                else:
                    cis = [4 * bb + 2 * half, 4 * bb + 2 * half + 1]
                for dc in range(2):
                    first = True
                    for ci in cis:
                        c = ci % NCH
                        for f in range(F):
                            nc.tensor.matmul(
                                MISC[:, numcol0 + dc : numcol0 + dc + 1],
                                x_nat[ci][:, f, dc * 128 : (dc + 1) * 128],
                                elog[bb][:, c * 8 + f : c * 8 + f + 1],
                                start=first,
                                stop=(ci == cis[-1] and f == F - 1),
                            )
                            first = False

            def num_copy(numcol0, outcol0, n=2):
                nc.vector.tensor_copy(
                    out=num_sb[:, outcol0 : outcol0 + n],
                    in_=MISC[:, numcol0 : numcol0 + n],
                )

            # ---- main software-pipelined slot loop ----
            # PE order per slot q: [pe-transpose half], score(q),
            # logits(q-2), [numerator of batch finished 2 slots ago]
            pe_t_pending = [
                (ci, h) for ci in range(NCHT) if ci in PE_T for h in (0, 1)
            ]

            for q in range(NSLOT):
                ci, g = divmod(q, 2)
                bb = ci // NCH
                # at most one PE transpose half-block per slot, emitted just
                # in time (a few slots before its score group) so the
                # in-order PE queue never heads-of-line-blocks on a load
                # that is still far out on the DMA device
                if pe_t_pending and 2 * pe_t_pending[0][0] - q <= 5:
                    tci, th = pe_t_pending.pop(0)
                    pe_transpose_half(tci, th)
                score_group(q)
                if q >= 2:
                    logits_group(q - 2)
                    qq = q - 2
                    if qq % 8 == 7 and qq // 8 < 3:
                        # batch bbq fully projected -> exp now, numerator
                        # two slots later (gives the ACT queue time)
                        exp_batch(qq // 8)
                if q >= 4 and (q - 4) % 8 == 7:
                    bbq = (q - 4) // 8
                    if bbq < 3:
                        num_block(bbq, None, 128 + 2 * bbq)
                        num_copy(128 + 2 * bbq, 2 * bbq)
                # last batch, first half: groups 24..27 projected once
                # logits(27) emitted at slot 29 -> exp half at 29, num at 31
                if q == 29:
                    exp_batch(3, half=0)
                if q == 31:
                    num_block(3, 0, 134)
                    num_copy(134, 6)

            # ---- tail: last two logit groups, second exp half, numerator;
            #      outputs already computed stream out early so only the
            #      last slivers sit on the critical path ----
            logits_group(30)
            nc.sync.dma_start(out=num_d[:, 0:8], in_=num_sb[:, 0:8])
            nc.scalar.dma_start(out=acc_d[:, 0:4], in_=acc_sb[:, 0:4])
            spin(4)
            logits_group(31)
            exp_batch(3, half=1)
            spin(5)
            num_block(3, 1, 136)
            num_copy(136, 8)

            nc.sync.dma_start(out=num_d[:, 8:10], in_=num_sb[:, 8:10])
            nc.scalar.dma_start(out=acc_d[:, 4:5], in_=acc_sb[:, 4:5])

    nc.compile()
    return nc


def _get_nc(use_bias=False):
    key = "nc_bias" if use_bias else "nc"
    if key not in _cache:
        _cache[key] = _build(use_bias)
    return _cache[key]


def kernel(inputs, W, b, V):
    sys.path.insert(0, _TRN_REPO)
    from concourse.bass_utils import run_bass_kernel_spmd

    inputs = np.ascontiguousarray(np.asarray(inputs, dtype=np.float32))
    W = np.ascontiguousarray(np.asarray(W, dtype=np.float32))
    b = np.ascontiguousarray(np.asarray(b, dtype=np.float32))
    V = np.ascontiguousarray(np.asarray(V, dtype=np.float32))

    # the fast build fuses tanh across both e-chunks, which requires b == 0
    # (guaranteed by the problem spec); fall back to a per-ec-bias build if
    # a nonzero bias ever shows up
    use_bias = bool(np.any(b != 0.0))
    nc = _get_nc(use_bias)

    in_maps = [
        {
            "inputs": inputs[i * B_LOC : (i + 1) * B_LOC],
            "W": W,
            "b": b,
            "V": V,
        }
        for i in range(N_CORES)
    ]

    trace = bool(int(os.environ.get("BENCH_TRACE", "0")))
    try:
        res = run_bass_kernel_spmd(
            nc, in_maps, core_ids=list(range(N_CORES)), trace=trace
        )
    except ModuleNotFoundError:
        res = run_bass_kernel_spmd(
            nc, in_maps, core_ids=list(range(N_CORES)), trace=False
        )
    _cache["last_exec_time_ns"] = res.exec_time_ns
    _cache["last_result"] = res
    outs = []
    for r in res.results:
        num = r["num"]          # [128, 10]
        acc = r["acc"]          # [128, 5]
        ctx = np.empty((B_LOC, D), dtype=np.float32)
        for bb in range(3):
            den = acc[:, bb].sum()
            ctx[bb, :128] = num[:, 2 * bb] / den
            ctx[bb, 128:] = num[:, 2 * bb + 1] / den
        den3 = acc[:, 3].sum() + acc[:, 4].sum()
        ctx[3, :128] = (num[:, 6] + num[:, 8]) / den3
        ctx[3, 128:] = (num[:, 7] + num[:, 9]) / den3
        outs.append(ctx)
    return np.concatenate(outs, axis=0)


# revision 14
# speedup vs baseline: 1.2725x; 1.1365x over previous
"""Trainium2 Bass kernel for attention pooling (nn_AttentionLayer).

Reference math (per batch b):
    score  = tanh(x @ W + b)        # [S, D]
    logits = score @ V              # [S, 1]
    attn   = softmax(logits, axis=S)
    out    = sum_s attn[s] * x[s]   # [D]

Sharding: data-parallel over batch across 8 NeuronCores (4 batches/core).
W/b/V replicated. No collectives.

Per-core dataflow (B_LOC=4, S=4096, D=256; seq chunks of SC=2048, folded
s = s0 + p*16 + f so each partition's source rows stay contiguous):
  1. SWDGE cast-DMA HBM->SBUF f32->bf16 into x_nat[p, f, d], issued as
     fold-half loads (quarters for chunk 0) and WAR-paced by the x pool
     depth - the DGE completion-sem rings are only 8 deep per class, so
     DMA issue must track consumption or recycling fences serialize
     everything.
  2. xT[d_low, (f,dc), s_low] built per 4-fold block, split between the
     DMA xbar (half-chunk transposes chained right behind their feeding
     load) and the PE (is_transpose matmuls into one PSUM bank, DVE
     copy-out), balancing the DMA and PE devices.
  3. scoreT[e, s] = W.T @ x.T on TensorE (W stationary, xT moving),
     PSUM pair tiles [128, 2(ec), 512].
  4. one ACT tanh per pair -> st bf16 (b==0 per the problem spec, so both
     ec halves merge into one instruction; a safe per-ec-bias build is
     compiled on demand if b is ever nonzero).
  5. logits via fat-stationary/skinny-moving matmuls: stationary =
     st[:, ec, 128-col block], moving = V chunk [128, 1] -> one PSUM
     column accumulated over ec. Lands logitsT directly in natural
     layout (PL region of the shared MISC psum bank) - no collect/
     scatter/transpose chain, and ldweights/1-col matmuls are ~free.
  6. one ACT exp per batch (PSUM -> elog bf16, accum_out -> denominator
     partials per partition; host sums the 128 partials).
  7. numerator with the same trick: stationary = x_nat[:, f, dc*128:...],
     moving = elog column [128, 1], accumulated into MISC NUM columns.
  8. outputs merge into one [128, 16] tensor; the bulk streams out while
     the last chunk's numerator still runs. Host does the divide.
The last batch's exp/numerator is split per chunk so half overlaps the
final score groups, shrinking the serial tail.
"""

import os
import sys

import numpy as np

_TRN_REPO = "/opt/trn_rl_repo"

B, S, D = 32, 4096, 256
N_CORES = 8
B_LOC = B // N_CORES          # 4 batches per core
SC = 2048                     # seq chunk
F = SC // 128                 # folds per chunk (16); s = s0 + p*F + f
NCH = S // SC                 # chunks per batch (2)
NCHT = B_LOC * NCH            # chunks per core (8)
NSLOT = 4 * NCHT              # score groups (512 seqs) per core (32)

# transpose blocks (chunk, group 0..3) handled on the PE instead of the
# DMA xbar; chunk 0/1 cover the pipeline fill, the late chunks offload
# the DMA device near the end
PE_BLOCKS = {
    (ci, g)
    for ci in (0, 1, 6, 7)
    for g in range(4)
} | {(5, 2), (5, 3)}
N_SPIN = 30                   # PE warm-up spin matmuls (256 cols each)

_cache = {}


def _build(use_bias=False):
    sys.path.insert(0, _TRN_REPO)
    import concourse.bacc as bacc
    import concourse.tile as tile
    from concourse import mybir

    f32 = mybir.dt.float32
    bf16 = mybir.dt.bfloat16

    nc = bacc.Bacc("TRN2", target_bir_lowering=False, debug=False)

    x_d = nc.dram_tensor("inputs", (B_LOC, S, D), f32, kind="ExternalInput")
    W_d = nc.dram_tensor("W", (D, D), f32, kind="ExternalInput")
    b_d = nc.dram_tensor("b", (D,), f32, kind="ExternalInput")
    V_d = nc.dram_tensor("V", (D, 1), f32, kind="ExternalInput")
    # merged output: cols 0-5 num b0..b2, 6-7 num b3 first half, 8-11 acc
    # [b0,b1,b2,b3a], 12-13 num b3 second half, 14 acc b3b, 15 pad
    out_d = nc.dram_tensor("out", (128, 16), f32, kind="ExternalOutput")

    with tile.TileContext(nc) as tc:
        with (
            tc.tile_pool(name="consts", bufs=1) as consts,
            tc.tile_pool(name="xpool", bufs=5) as xpool,
            tc.tile_pool(name="xtpool", bufs=4) as xtpool,
            tc.tile_pool(name="stpool", bufs=4) as stpool,
            tc.tile_pool(name="smalls", bufs=1) as smalls,
            tc.tile_pool(name="pspool", bufs=3, space="PSUM") as pspool,
            tc.tile_pool(name="trpool", bufs=1, space="PSUM") as trpool,
            tc.tile_pool(name="miscpool", bufs=1, space="PSUM") as miscpool,
        ):
            # ---- dependency-free prologue first: DVE memsets + Pool
            #      identity build, so PE transposes / warm-up spins aren't
            #      stuck behind loads or casts in those queues ----
            ones_sb = consts.tile([128, 128], bf16)
            nc.vector.memset(ones_sb, 1.0)
            dummy_sb = consts.tile([128, 2], bf16)
            nc.vector.memset(dummy_sb, 0.0)
            dummy_mov = consts.tile([128, 256], bf16)
            nc.vector.memset(dummy_mov, 0.0)
            zero_bias = consts.tile([128, 1], f32)
            nc.vector.memset(zero_bias, 0.0)
            I_sb = consts.tile([128, 128], bf16)
            nc.gpsimd.affine_select(
                out=I_sb,
                in_=ones_sb,
                pattern=[[-1, 128]],
                compare_op=mybir.AluOpType.is_equal,
                fill=0.0,
                base=0,
                channel_multiplier=1,
            )

            # params on HWDGE/SP (transfers overlap the first x loads)
            W_f = consts.tile([128, 2, D], f32)
            nc.sync.dma_start(
                out=W_f, in_=W_d[:, :].rearrange("(dc p) e -> p dc e", p=128)
            )
            V_f = consts.tile([128, 2], f32)
            nc.sync.dma_start(
                out=V_f, in_=V_d[:, :].rearrange("(ec p) o -> p (ec o)", p=128)
            )
            b_sb = consts.tile([128, 2], f32)
            nc.sync.dma_start(
                out=b_sb, in_=b_d[:].rearrange("(ec p) -> p ec", p=128)
            )
            W_sb = consts.tile([128, 2, D], bf16)
            nc.vector.tensor_copy(out=W_sb, in_=W_f)
            V_sb = consts.tile([128, 2], bf16)
            nc.vector.tensor_copy(out=V_sb, in_=V_f)

            x_nat = {}
            xT = {}

            def load_part(ci, f0, f1):
                # fold range [f0, f1) of chunk ci; per-partition source rows
                # stay contiguous (s = s0 + p*F + f)
                bb, c = divmod(ci, NCH)
                s0 = c * SC
                if ci not in x_nat:
                    x_nat[ci] = xpool.tile([128, F, D], bf16, name="x_nat", tag="x")
                src = x_d[bb, s0 : s0 + SC, :].rearrange("(p f) d -> p f d", p=128)
                nc.gpsimd.dma_start(
                    out=x_nat[ci][:, f0:f1, :], in_=src[:, f0:f1, :]
                )

            def dma_transpose_half(ci, h):
                # folds [8h, 8h+8) -> xT rows [16h, 16h+16)
                if ci not in xT:
                    xT[ci] = xtpool.tile([128, 2 * F, 128], bf16, name="xT", tag="xT")
                nc.sync.dma_start(
                    out=xT[ci][:, 16 * h : 16 * h + 16, :],
                    in_=x_nat[ci][:, 8 * h : 8 * h + 8, :],
                    transpose=True,
                )

            def pe_transpose_block(ci, g):
                # folds [4g, 4g+4) -> 8 [128,128] tiles -> 1 psum bank
                if ci not in xT:
                    xT[ci] = xtpool.tile([128, 2 * F, 128], bf16, name="xT", tag="xT")
                tr = trpool.tile([128, 1024], bf16, name="tr", tag="tr")
                for t in range(8):
                    f = 4 * g + t // 2
                    dc = t % 2
                    nc.tensor.transpose(
                        out=tr[:, t * 128 : (t + 1) * 128],
                        in_=x_nat[ci][:, f, dc * 128 : (dc + 1) * 128],
                        identity=I_sb,
                    )
                nc.vector.tensor_copy(
                    out=xT[ci][:, 8 * g : 8 * g + 8, :],
                    in_=tr.rearrange("p (a b) -> p a b", a=8),
                )

            def issue_chunk_dmas(ci):
                # fold-half loads (quarters for chunk 0), each DMA-xbar
                # half-transpose chained right behind the load that feeds it
                # so the shared DMA device serves them in need-order
                if ci == 0:
                    parts = [(0, 4), (4, 8), (8, 12), (12, 16)]
                else:
                    parts = [(0, 8), (8, 16)]
                for f0, f1 in parts:
                    load_part(ci, f0, f1)
                    if f1 % 8 == 0:
                        h = f1 // 8 - 1
                        if (ci, 2 * h) not in PE_BLOCKS:
                            dma_transpose_half(ci, h)

            # shared psum bank: PL logits cols 0..127, NUM cols 128..137,
            # spin target cols 256..511
            MISC = miscpool.tile([128, 512], f32, name="MISC")

            def spin(n):
                for _ in range(n):
                    nc.tensor.matmul(
                        MISC[0:2, 256:512],
                        dummy_sb,
                        dummy_mov,
                        start=True,
                        stop=True,
                    )

            issue_chunk_dmas(0)
            issue_chunk_dmas(1)
            issue_chunk_dmas(2)

            spin(N_SPIN)

            # ---- outputs / softmax state ----
            out_sb = smalls.tile([128, 16], f32, name="out_sb")
            elog = {}
            st_tiles = {}

            def score_group(q):
                ci, g = divmod(q, 4)
                ps = pspool.tile([128, 2, 512], f32, name="ps", tag="ps")
                xv = xT[ci].rearrange("p (f dc) s -> p f dc s", dc=2)
                for ec in range(2):
                    for dc in range(2):
                        nc.tensor.matmul(
                            ps[:, ec, :],
                            W_sb[:, dc, ec * 128 : (ec + 1) * 128],
                            xv[:, 4 * g : 4 * g + 4, dc, :],
                            start=(dc == 0),
                            stop=(dc == 1),
                        )
                st = stpool.tile([128, 2, 512], bf16, name="st", tag="st")
                if use_bias:
                    for ec in range(2):
                        nc.scalar.activation(
                            out=st[:, ec, :],
                            in_=ps[:, ec, :],
                            func=mybir.ActivationFunctionType.Tanh,
                            bias=b_sb[:, ec : ec + 1],
                            scale=1.0,
                        )
                else:
                    nc.scalar.activation(
                        out=st,
                        in_=ps,
                        func=mybir.ActivationFunctionType.Tanh,
                        bias=zero_bias[:, 0:1],
                        scale=1.0,
                    )
                st_tiles[q] = st

            def logits_group(q):
                ci, g = divmod(q, 4)
                bb, c = divmod(ci, NCH)
                st = st_tiles.pop(q)
                for k in range(4):
                    col = bb * 32 + c * 16 + g * 4 + k
                    for ec in range(2):
                        nc.tensor.matmul(
                            MISC[:, col : col + 1],
                            st[:, ec, k * 128 : (k + 1) * 128],
                            V_sb[:, ec : ec + 1],
                            start=(ec == 0),
                            stop=(ec == 1),
                        )

            def exp_batch(bb, half=None):
                if bb not in elog:
                    elog[bb] = smalls.tile(
                        [128, 32], bf16, name="elog", tag="elog", bufs=2
                    )
                if half is None:
                    src = MISC[:, bb * 32 : bb * 32 + 32]
                    dst = elog[bb]
                    acc = out_sb[:, 8 + bb : 9 + bb]
                else:
                    src = MISC[:, bb * 32 + 16 * half : bb * 32 + 16 * (half + 1)]
                    dst = elog[bb][:, 16 * half : 16 * (half + 1)]
                    acc = out_sb[:, 11 + 3 * half : 12 + 3 * half]
                nc.scalar.activation(
                    out=dst,
                    in_=src,
                    func=mybir.ActivationFunctionType.Exp,
                    accum_out=acc,
                )

            def num_block(bb, half, numcol0):
                # accumulate numerator into MISC cols numcol0 (dc=0) /
                # numcol0+1 (dc=1); half=None covers the whole batch,
                # half=0/1 one chunk each (last batch; merged on host)
                if half is None:
                    cis = [NCH * bb, NCH * bb + 1]
                else:
                    cis = [NCH * bb + half]
                for dc in range(2):
                    first = True
                    for ci in cis:
                        c = ci % NCH
                        for f in range(F):
                            nc.tensor.matmul(
                                MISC[:, numcol0 + dc : numcol0 + dc + 1],
                                x_nat[ci][:, f, dc * 128 : (dc + 1) * 128],
                                elog[bb][:, c * 16 + f : c * 16 + f + 1],
                                start=first,
                                stop=(ci == cis[-1] and f == F - 1),
                            )
                            first = False

            def num_copy(numcol0, outcol0, n=2):
                nc.vector.tensor_copy(
                    out=out_sb[:, outcol0 : outcol0 + n],
                    in_=MISC[:, numcol0 : numcol0 + n],
                )

            # ---- main software-pipelined slot loop ----
            pe_t_pending = sorted(PE_BLOCKS)

            for q in range(NSLOT):
                ci, g = divmod(q, 4)
                # keep loads ~3 chunks ahead (plus WAR pacing from xpool)
                if g == 0 and ci + 3 < NCHT:
                    issue_chunk_dmas(ci + 3)
                # at most one PE transpose block per slot, just in time so
                # the in-order PE queue never blocks on a far-out load
                if pe_t_pending:
                    tci, tg = pe_t_pending[0]
                    if 4 * tci + tg <= q + 5:
                        pe_t_pending.pop(0)
                        pe_transpose_block(tci, tg)
                score_group(q)
                if q >= 2:
                    logits_group(q - 2)
                    qq = q - 2
                    if qq % 8 == 7 and qq // 8 < 3:
                        exp_batch(qq // 8)
                if q >= 4 and (q - 4) % 8 == 7:
                    bbq = (q - 4) // 8
                    if bbq < 3:
                        num_block(bbq, None, 128 + 2 * bbq)
                        num_copy(128 + 2 * bbq, 2 * bbq)
                # last batch, first chunk: its 4 groups are projected once
                # logits(27) lands at slot 29 -> exp half there, num at 31
                if q == 29:
                    exp_batch(3, half=0)
                if q == 31:
                    num_block(3, 0, 134)
                    num_copy(134, 6)

            # ---- tail: last two logit groups, second exp half, numerator;
            #      the bulk of the output streams out under the tail ----
            logits_group(30)
            nc.sync.dma_start(out=out_d[:, 0:12], in_=out_sb[:, 0:12])
            spin(4)
            logits_group(31)
            exp_batch(3, half=1)
            spin(5)
            num_block(3, 1, 136)
            num_copy(136, 12)

            nc.scalar.dma_start(out=out_d[:, 12:16], in_=out_sb[:, 12:16])

    nc.compile()
    return nc


def _get_nc(use_bias=False):
    key = "nc_bias" if use_bias else "nc"
    if key not in _cache:
        _cache[key] = _build(use_bias)
    return _cache[key]


def kernel(inputs, W, b, V):
    sys.path.insert(0, _TRN_REPO)
    from concourse.bass_utils import run_bass_kernel_spmd

    inputs = np.ascontiguousarray(np.asarray(inputs, dtype=np.float32))
    W = np.ascontiguousarray(np.asarray(W, dtype=np.float32))
    b = np.ascontiguousarray(np.asarray(b, dtype=np.float32))
    V = np.ascontiguousarray(np.asarray(V, dtype=np.float32))

    # the fast build fuses tanh across both e-chunks, which requires b == 0
    # (guaranteed by the problem spec); fall back to a per-ec-bias build if
    # a nonzero bias ever shows up
    use_bias = bool(np.any(b != 0.0))
    nc = _get_nc(use_bias)

    in_maps = [
        {
            "inputs": inputs[i * B_LOC : (i + 1) * B_LOC],
            "W": W,
            "b": b,
            "V": V,
        }
        for i in range(N_CORES)
    ]

    trace = bool(int(os.environ.get("BENCH_TRACE", "0")))
    try:
        res = run_bass_kernel_spmd(
            nc, in_maps, core_ids=list(range(N_CORES)), trace=trace
        )
    except ModuleNotFoundError:
        res = run_bass_kernel_spmd(
            nc, in_maps, core_ids=list(range(N_CORES)), trace=False
        )
    _cache["last_exec_time_ns"] = res.exec_time_ns
    _cache["last_result"] = res
    outs = []
    for r in res.results:
        o = r["out"]            # [128, 16]
        ctx = np.empty((B_LOC, D), dtype=np.float32)
        for bb in range(3):
            den = o[:, 8 + bb].sum()
            ctx[bb, :128] = o[:, 2 * bb] / den
            ctx[bb, 128:] = o[:, 2 * bb + 1] / den
        den3 = o[:, 11].sum() + o[:, 14].sum()
        ctx[3, :128] = (o[:, 6] + o[:, 12]) / den3
        ctx[3, 128:] = (o[:, 7] + o[:, 13]) / den3
        outs.append(ctx)
    return np.concatenate(outs, axis=0)


# revision 15
# speedup vs baseline: 1.4154x; 1.1124x over previous
"""Trainium2 Bass kernel for attention pooling (nn_AttentionLayer).

Reference math (per batch b):
    score  = tanh(x @ W + b)        # [S, D]
    logits = score @ V              # [S, 1]
    attn   = softmax(logits, axis=S)
    out    = sum_s attn[s] * x[s]   # [D]

Sharding: data-parallel over batch across 8 NeuronCores (4 batches/core).
W/b/V replicated. No collectives.

Per-core dataflow (B_LOC=4, S=4096, D=256; seq chunks of SC=2048, folded
s = s0 + p*16 + f so each partition's source rows stay contiguous):
  1. SWDGE cast-DMA HBM->SBUF f32->bf16 into x_nat[p, f, d], issued as
     fold-half loads (quarters for chunk 0) and WAR-paced by the x pool
     depth - the DGE completion-sem rings are only 8 deep per class, so
     DMA issue must track consumption or recycling fences serialize
     everything.
  2. xT[d_low, (f,dc), s_low] built per 4-fold block, split between the
     DMA xbar (half-chunk transposes chained right behind their feeding
     load) and the PE (is_transpose matmuls into one PSUM bank, DVE
     copy-out), balancing the DMA and PE devices.
  3. scoreT[e, s] = W.T @ x.T on TensorE (W stationary, xT moving),
     PSUM pair tiles [128, 2(ec), 512].
  4. one ACT tanh per pair -> st bf16 (b==0 per the problem spec, so both
     ec halves merge into one instruction; a safe per-ec-bias build is
     compiled on demand if b is ever nonzero).
  5. logits via fat-stationary/skinny-moving matmuls: stationary =
     st[:, ec, 128-col block], moving = V chunk [128, 1] -> one PSUM
     column accumulated over ec. Lands logitsT directly in natural
     layout (PL region of the shared MISC psum bank) - no collect/
     scatter/transpose chain, and ldweights/1-col matmuls are ~free.
  6. one ACT exp per batch (PSUM -> elog bf16, accum_out -> denominator
     partials per partition; host sums the 128 partials).
  7. numerator with the same trick: stationary = x_nat[:, f, dc*128:...],
     moving = elog column [128, 1], accumulated into MISC NUM columns.
  8. outputs merge into one [128, 16] tensor; the bulk streams out while
     the last chunk's numerator still runs. Host does the divide.
The last batch's exp/numerator is split per chunk so half overlaps the
final score groups, shrinking the serial tail.
"""

import os
import sys

import numpy as np

_TRN_REPO = "/opt/trn_rl_repo"

B, S, D = 32, 4096, 256
N_CORES = 8
B_LOC = B // N_CORES          # 4 batches per core
SC = 2048                     # seq chunk
F = SC // 128                 # folds per chunk (16); s = s0 + p*F + f
NCH = S // SC                 # chunks per batch (2)
NCHT = B_LOC * NCH            # chunks per core (8)
NSLOT = 4 * NCHT              # score groups (512 seqs) per core (32)

# transpose blocks (chunk, group 0..3) handled on the PE instead of the
# DMA xbar; chunk 0/1 cover the pipeline fill, the late chunks offload
# the DMA device near the end
PE_BLOCKS = {
    (ci, g)
    for ci in (0, 1)
    for g in range(4)
} | {(ci, g) for ci in range(2, NCHT) for g in (0, 1)}
N_SPIN = 18                   # PE warm-up spin matmuls (256 cols each)

_cache = {}


def _build(use_bias=False):
    sys.path.insert(0, _TRN_REPO)
    import concourse.bacc as bacc
    import concourse.tile as tile
    from concourse import mybir

    f32 = mybir.dt.float32
    bf16 = mybir.dt.bfloat16

    nc = bacc.Bacc("TRN2", target_bir_lowering=False, debug=False)

    x_d = nc.dram_tensor("inputs", (B_LOC, S, D), f32, kind="ExternalInput")
    W_d = nc.dram_tensor("W", (D, D), f32, kind="ExternalInput")
    b_d = nc.dram_tensor("b", (D,), f32, kind="ExternalInput")
    V_d = nc.dram_tensor("V", (D, 1), f32, kind="ExternalInput")
    # merged output: cols 0-5 num b0..b2, 6-7 num chunk6, 8-10 acc b0..b2,
    # 11 acc chunk6, 12-13 num chunk7a, 14 acc chunk7a, 16-17 num chunk7b,
    # 18 acc chunk7b, rest pad
    out_d = nc.dram_tensor("out", (128, 20), f32, kind="ExternalOutput")

    with tile.TileContext(nc) as tc:
        with (
            tc.tile_pool(name="consts", bufs=1) as consts,
            tc.tile_pool(name="xpool", bufs=6) as xpool,
            tc.tile_pool(name="xtpool", bufs=4) as xtpool,
            tc.tile_pool(name="stpool", bufs=4) as stpool,
            tc.tile_pool(name="smalls", bufs=1) as smalls,
            tc.tile_pool(name="pspool", bufs=2, space="PSUM") as pspool,
            tc.tile_pool(name="trpool", bufs=2, space="PSUM") as trpool,
            tc.tile_pool(name="miscpool", bufs=1, space="PSUM") as miscpool,
        ):
            # ---- dependency-free prologue first: DVE memsets + Pool
            #      identity build, so PE transposes / warm-up spins aren't
            #      stuck behind loads or casts in those queues ----
            ones_sb = consts.tile([128, 128], bf16)
            nc.vector.memset(ones_sb, 1.0)
            dummy_sb = consts.tile([128, 2], bf16)
            nc.vector.memset(dummy_sb, 0.0)
            dummy_mov = consts.tile([128, 256], bf16)
            nc.vector.memset(dummy_mov, 0.0)
            zero_bias = consts.tile([128, 1], f32)
            nc.vector.memset(zero_bias, 0.0)
            I_sb = consts.tile([128, 128], bf16)
            nc.gpsimd.affine_select(
                out=I_sb,
                in_=ones_sb,
                pattern=[[-1, 128]],
                compare_op=mybir.AluOpType.is_equal,
                fill=0.0,
                base=0,
                channel_multiplier=1,
            )

            # params on HWDGE/SP (transfers overlap the first x loads)
            W_f = consts.tile([128, 2, D], f32)
            nc.sync.dma_start(
                out=W_f, in_=W_d[:, :].rearrange("(dc p) e -> p dc e", p=128)
            )
            V_f = consts.tile([128, 2], f32)
            nc.sync.dma_start(
                out=V_f, in_=V_d[:, :].rearrange("(ec p) o -> p (ec o)", p=128)
            )
            b_sb = consts.tile([128, 2], f32)
            nc.sync.dma_start(
                out=b_sb, in_=b_d[:].rearrange("(ec p) -> p ec", p=128)
            )
            W_sb = consts.tile([128, 2, D], bf16)
            nc.vector.tensor_copy(out=W_sb, in_=W_f)
            V_sb = consts.tile([128, 2], bf16)
            nc.vector.tensor_copy(out=V_sb, in_=V_f)

            x_nat = {}
            xT = {}

            def load_part(ci, f0, f1):
                # fold range [f0, f1) of chunk ci; per-partition source rows
                # stay contiguous (s = s0 + p*F + f)
                bb, c = divmod(ci, NCH)
                s0 = c * SC
                if ci not in x_nat:
                    x_nat[ci] = xpool.tile([128, F, D], bf16, name="x_nat", tag="x")
                src = x_d[bb, s0 : s0 + SC, :].rearrange("(p f) d -> p f d", p=128)
                nc.gpsimd.dma_start(
                    out=x_nat[ci][:, f0:f1, :], in_=src[:, f0:f1, :]
                )

            def dma_transpose_half(ci, h):
                # folds [8h, 8h+8) -> xT rows [16h, 16h+16)
                if ci not in xT:
                    xT[ci] = xtpool.tile([128, 2 * F, 128], bf16, name="xT", tag="xT")
                nc.sync.dma_start(
                    out=xT[ci][:, 16 * h : 16 * h + 16, :],
                    in_=x_nat[ci][:, 8 * h : 8 * h + 8, :],
                    transpose=True,
                )

            def pe_transpose_block(ci, g):
                # folds [4g, 4g+4) -> 8 [128,128] tiles -> 1 psum bank
                if ci not in xT:
                    xT[ci] = xtpool.tile([128, 2 * F, 128], bf16, name="xT", tag="xT")
                tr = trpool.tile([128, 1024], bf16, name="tr", tag="tr")
                for t in range(8):
                    f = 4 * g + t // 2
                    dc = t % 2
                    nc.tensor.transpose(
                        out=tr[:, t * 128 : (t + 1) * 128],
                        in_=x_nat[ci][:, f, dc * 128 : (dc + 1) * 128],
                        identity=I_sb,
                    )
                nc.vector.tensor_copy(
                    out=xT[ci][:, 8 * g : 8 * g + 8, :],
                    in_=tr.rearrange("p (a b) -> p a b", a=8),
                )

            def issue_chunk_dmas(ci):
                # fold-half loads (quarters for chunk 0), each DMA-xbar
                # half-transpose chained right behind the load that feeds it
                # so the shared DMA device serves them in need-order
                if ci == 0:
                    parts = [(0, 4), (4, 8), (8, 12), (12, 16)]
                else:
                    parts = [(0, 8), (8, 16)]
                for f0, f1 in parts:
                    load_part(ci, f0, f1)
                    if f1 % 8 == 0:
                        h = f1 // 8 - 1
                        if (ci, 2 * h) not in PE_BLOCKS:
                            dma_transpose_half(ci, h)

            # shared psum bank: PL logits cols 0..127, NUM cols 128..137,
            # spin target cols 256..511
            MISC = miscpool.tile([128, 512], f32, name="MISC")

            def spin(n):
                for _ in range(n):
                    nc.tensor.matmul(
                        MISC[0:2, 256:512],
                        dummy_sb,
                        dummy_mov,
                        start=True,
                        stop=True,
                    )

            issue_chunk_dmas(0)
            issue_chunk_dmas(1)
            issue_chunk_dmas(2)

            spin(N_SPIN)

            # ---- outputs / softmax state ----
            out_sb = smalls.tile([128, 20], f32, name="out_sb")
            elog = {}
            st_tiles = {}

            def score_group(q):
                ci, g = divmod(q, 4)
                ps = pspool.tile([128, 2, 512], f32, name="ps", tag="ps")
                xv = xT[ci].rearrange("p (f dc) s -> p f dc s", dc=2)
                for ec in range(2):
                    for dc in range(2):
                        nc.tensor.matmul(
                            ps[:, ec, :],
                            W_sb[:, dc, ec * 128 : (ec + 1) * 128],
                            xv[:, 4 * g : 4 * g + 4, dc, :],
                            start=(dc == 0),
                            stop=(dc == 1),
                        )
                st = stpool.tile([128, 2, 512], bf16, name="st", tag="st")
                if use_bias:
                    for ec in range(2):
                        nc.scalar.activation(
                            out=st[:, ec, :],
                            in_=ps[:, ec, :],
                            func=mybir.ActivationFunctionType.Tanh,
                            bias=b_sb[:, ec : ec + 1],
                            scale=1.0,
                        )
                else:
                    nc.scalar.activation(
                        out=st,
                        in_=ps,
                        func=mybir.ActivationFunctionType.Tanh,
                        bias=zero_bias[:, 0:1],
                        scale=1.0,
                    )
                st_tiles[q] = st

            def logits_group(q):
                ci, g = divmod(q, 4)
                bb, c = divmod(ci, NCH)
                st = st_tiles.pop(q)
                for k in range(4):
                    col = bb * 32 + c * 16 + g * 4 + k
                    for ec in range(2):
                        nc.tensor.matmul(
                            MISC[:, col : col + 1],
                            st[:, ec, k * 128 : (k + 1) * 128],
                            V_sb[:, ec : ec + 1],
                            start=(ec == 0),
                            stop=(ec == 1),
                        )

            def exp_piece(bb, c0, c1, acc_col):
                # exp over elog cols [c0, c1) of batch bb, denominator
                # partials accumulated into out_sb[:, acc_col]
                if bb not in elog:
                    elog[bb] = smalls.tile(
                        [128, 32], bf16, name="elog", tag="elog", bufs=2
                    )
                nc.scalar.activation(
                    out=elog[bb][:, c0:c1],
                    in_=MISC[:, bb * 32 + c0 : bb * 32 + c1],
                    func=mybir.ActivationFunctionType.Exp,
                    accum_out=out_sb[:, acc_col : acc_col + 1],
                )

            def num_block(bb, pieces, numcol0):
                # accumulate numerator over (chunk, f0, f1) pieces into
                # MISC cols numcol0 (dc=0) / numcol0+1 (dc=1)
                for dc in range(2):
                    first = True
                    for pi, (ci, f0, f1) in enumerate(pieces):
                        c = ci % NCH
                        last_piece = pi == len(pieces) - 1
                        for f in range(f0, f1):
                            nc.tensor.matmul(
                                MISC[:, numcol0 + dc : numcol0 + dc + 1],
                                x_nat[ci][:, f, dc * 128 : (dc + 1) * 128],
                                elog[bb][:, c * 16 + f : c * 16 + f + 1],
                                start=first,
                                stop=(last_piece and f == f1 - 1),
                            )
                            first = False

            def num_copy(numcol0, outcol0, n=2):
                nc.vector.tensor_copy(
                    out=out_sb[:, outcol0 : outcol0 + n],
                    in_=MISC[:, numcol0 : numcol0 + n],
                )

            # ---- main software-pipelined slot loop ----
            pe_t_pending = sorted(PE_BLOCKS)

            for q in range(NSLOT):
                ci, g = divmod(q, 4)
                # keep loads ~3 chunks ahead (plus WAR pacing from xpool)
                if g == 0 and ci + 3 < NCHT:
                    issue_chunk_dmas(ci + 3)
                # at most one PE transpose block per slot, just in time so
                # the in-order PE queue never blocks on a far-out load
                if pe_t_pending:
                    tci, tg = pe_t_pending[0]
                    if 4 * tci + tg <= q + 5:
                        pe_t_pending.pop(0)
                        pe_transpose_block(tci, tg)
                score_group(q)
                if q >= 2:
                    logits_group(q - 2)
                    qq = q - 2
                    if qq % 8 == 7 and qq // 8 < 3:
                        exp_piece(qq // 8, 0, 32, 8 + qq // 8)
                if q >= 4 and (q - 4) % 8 == 7:
                    bbq = (q - 4) // 8
                    if bbq < 3:
                        num_block(
                            bbq,
                            [(NCH * bbq, 0, F), (NCH * bbq + 1, 0, F)],
                            128 + 2 * bbq,
                        )
                        num_copy(128 + 2 * bbq, 2 * bbq)
                # last batch: chunk 6 projected once logits(27) lands at
                # slot 29 -> exp there, numerator at 31; chunk 7's first
                # half exps at 31 so its numerator overlaps the epilogue
                if q == 29:
                    exp_piece(3, 0, 16, 11)
                if q == 31:
                    num_block(3, [(6, 0, F)], 134)
                    num_copy(134, 6)
                    exp_piece(3, 16, 24, 14)

            # ---- tail: last two logit groups, the remaining exp pieces
            #      and numerator slivers; the bulk of the output streams
            #      out under the tail ----
            logits_group(30)
            nc.sync.dma_start(out=out_d[:, 0:12], in_=out_sb[:, 0:12])
            num_block(3, [(7, 0, 8)], 136)
            num_copy(136, 12)
            spin(3)
            logits_group(31)
            exp_piece(3, 24, 32, 18)
            spin(4)
            num_block(3, [(7, 8, F)], 138)
            num_copy(138, 16)

            nc.scalar.dma_start(out=out_d[:, 12:20], in_=out_sb[:, 12:20])

    nc.compile()
    return nc


def _get_nc(use_bias=False):
    key = "nc_bias" if use_bias else "nc"
    if key not in _cache:
        _cache[key] = _build(use_bias)
    return _cache[key]


def kernel(inputs, W, b, V):
    sys.path.insert(0, _TRN_REPO)
    from concourse.bass_utils import run_bass_kernel_spmd

    inputs = np.ascontiguousarray(np.asarray(inputs, dtype=np.float32))
    W = np.ascontiguousarray(np.asarray(W, dtype=np.float32))
    b = np.ascontiguousarray(np.asarray(b, dtype=np.float32))
    V = np.ascontiguousarray(np.asarray(V, dtype=np.float32))

    # the fast build fuses tanh across both e-chunks, which requires b == 0
    # (guaranteed by the problem spec); fall back to a per-ec-bias build if
    # a nonzero bias ever shows up
    use_bias = bool(np.any(b != 0.0))
    nc = _get_nc(use_bias)

    in_maps = [
        {
            "inputs": inputs[i * B_LOC : (i + 1) * B_LOC],
            "W": W,
            "b": b,
            "V": V,
        }
        for i in range(N_CORES)
    ]

    trace = bool(int(os.environ.get("BENCH_TRACE", "0")))
    try:
        res = run_bass_kernel_spmd(
            nc, in_maps, core_ids=list(range(N_CORES)), trace=trace
        )
    except ModuleNotFoundError:
        res = run_bass_kernel_spmd(
            nc, in_maps, core_ids=list(range(N_CORES)), trace=False
        )
    _cache["last_exec_time_ns"] = res.exec_time_ns
    _cache["last_result"] = res
    outs = []
    for r in res.results:
        o = r["out"]            # [128, 20]
        ctx = np.empty((B_LOC, D), dtype=np.float32)
        for bb in range(3):
            den = o[:, 8 + bb].sum()
            ctx[bb, :128] = o[:, 2 * bb] / den
            ctx[bb, 128:] = o[:, 2 * bb + 1] / den
        den3 = o[:, 11].sum() + o[:, 14].sum() + o[:, 18].sum()
        ctx[3, :128] = (o[:, 6] + o[:, 12] + o[:, 16]) / den3
        ctx[3, 128:] = (o[:, 7] + o[:, 13] + o[:, 17]) / den3
        outs.append(ctx)
    return np.concatenate(outs, axis=0)
